# revision 17
# baseline (speedup 1.0000x reference)
"""Trainium2 Bass kernel for nn_CognitiveNetwork (16-cell LSTM message-passing net).

Strategy
--------
* Expert-parallel over the C=16 cells: 2 cells per NeuronCore; all weights
  resident in SBUF (bf16).
* Transposed dataflow: activations live as [H, B] (H on partitions).
* Per-step cell-mean y: 256KB bf16 AllReduce across the 8 cores; y feeds the
  next step's external injection (x_t + 0.3*y).
* Critical-path restructure vs v1:
  - The Whh*h half of the LSTM gates only depends on the previous h, so it is
    computed at the END of step t (right after h_new) and spilled to SBUF as
    bf16 (with the gate bias folded in).  Those 128 matmuls execute during the
    AllReduce window, keeping the PE busy (HAM stays at K=8/8) and off the
    next step's critical path.  Step t+1 re-injects the spill into PSUM with
    an identity matmul and accumulates the Wih part on top.
  - rstd = 1/sqrt(var+eps) computed entirely on DVE/GPSIMD via the
    fast-inverse-sqrt bit trick + Newton steps: no ACT Sqrt, so the ACT
    engine never leaves the sigmoid/tanh table set (saves 2 table reloads =
    ~5.3us per step).
  - Gate activations batched in pairs (N=512) by function; LN-smalls chains
    for the two cells run concurrently on DVE (cell 0) and GPSIMD (cell 1).
* Embedding gather + input projection sharded over cores by timestep, then
  one AllGather of xs^T [T, H, B] bf16 (unchanged).
"""

import os
import sys

sys.path.insert(0, "/opt/trn_rl_repo")

import numpy as np
import ml_dtypes

from concourse import bass, bacc, mybir, tile
from concourse.bass_utils import run_bass_kernel_spmd

BF16 = ml_dtypes.bfloat16

# Problem constants (hardcoded per contract).
V, E, H, C = 50257, 256, 512, 16
B, T = 256, 128
LN_EPS = 1e-5

NCORES = 8
CPC = C // NCORES        # cells per core = 2
HC = H // 128            # h chunks = 4
EC = E // 128            # e chunks = 2
GC = (4 * H) // 128      # gate chunks = 16
TLOC = T // NCORES       # timesteps gathered per core = 16
NGRP = TLOC // 2         # preamble groups per core (2 t's = 512 tokens each)

NEWTON_ITERS = 1

F32 = mybir.dt.float32
BF = mybir.dt.bfloat16
I32 = mybir.dt.int32
AF = mybir.ActivationFunctionType
ALU = mybir.AluOpType
RG = [list(range(NCORES))]


def _pack_lhsT(w: np.ndarray) -> np.ndarray:
    """Pack [K, M] weight into SBUF lhsT layout [128, (K/128)*(M/128)*128]."""
    K, M = w.shape
    kc, mc = K // 128, M // 128
    return np.ascontiguousarray(
        w.reshape(kc, 128, mc, 128).transpose(1, 0, 2, 3).reshape(128, kc * mc * 128)
    )


def _pack_bias(b: np.ndarray) -> np.ndarray:
    """[n, M] -> [128, n*(M/128)]: column n*idx... (cell-major, chunk-minor)."""
    n, M = b.shape
    mc = M // 128
    return np.ascontiguousarray(
        b.reshape(n, mc, 128).transpose(2, 0, 1).reshape(128, n * mc)
    )


def build_program(t_steps: int = T):
    nc = bacc.Bacc(
        "TRN2",
        target_bir_lowering=False,
        debug=False,
        num_devices=NCORES,
    )

    # ---- I/O -------------------------------------------------------------
    emb_d = nc.declare_dram_parameter("emb", [V, E], BF, isOutput=False)
    tok_d = nc.declare_dram_parameter("tok", [NGRP * 4, 128, 1], I32, isOutput=False)
    wproj_d = nc.declare_dram_parameter("wproj", [128, EC * HC * 128], BF, isOutput=False)
    bproj_d = nc.declare_dram_parameter("bproj", [128, HC], F32, isOutput=False)
    wp_d = nc.declare_dram_parameter("wp", [128, CPC * HC * HC * 128], BF, isOutput=False)
    wih_d = nc.declare_dram_parameter("wih", [128, CPC * HC * GC * 128], BF, isOutput=False)
    whh_d = nc.declare_dram_parameter("whh", [128, CPC * HC * GC * 128], BF, isOutput=False)
    wa_d = nc.declare_dram_parameter("wa", [128, CPC * HC * HC * 128], BF, isOutput=False)
    w1n_d = nc.declare_dram_parameter("w1n", [1, CPC * GC * 128], BF, isOutput=False)
    bp_d = nc.declare_dram_parameter("bp", [128, CPC * HC], F32, isOutput=False)
    bg_d = nc.declare_dram_parameter("bg", [128, CPC * GC], F32, isOutput=False)
    ba_d = nc.declare_dram_parameter("ba", [128, CPC * HC], F32, isOutput=False)
    gsc_d = nc.declare_dram_parameter("gsc", [128, CPC], F32, isOutput=False)
    ident_d = nc.declare_dram_parameter("ident", [128, 128], BF, isOutput=False)
    out_d = nc.declare_dram_parameter("out", [t_steps, H, B], BF, isOutput=True)

    with tile.TileContext(nc) as tc:
        with (
            tc.tile_pool(name="wpool", bufs=1) as wpool,
            tc.tile_pool(name="state", bufs=1) as state,
            tc.tile_pool(name="dram", bufs=1, space="DRAM") as dpool1,
            tc.tile_pool(name="dramr", bufs=2, space="DRAM") as dpool2,
        ):
            # ---- resident SBUF tensors ----------------------------------
            wp_sb = wpool.tile([128, CPC * HC * HC * 128], BF, name="wp_sb")
            wih_sb = wpool.tile([128, CPC * HC * GC * 128], BF, name="wih_sb")
            whh_sb = wpool.tile([128, CPC * HC * GC * 128], BF, name="whh_sb")
            wa_sb = wpool.tile([128, CPC * HC * HC * 128], BF, name="wa_sb")
            w1n_sb = wpool.tile([1, CPC * GC * 128], BF, name="w1n_sb")
            bp_sb = wpool.tile([128, CPC * HC], F32, name="bp_sb")
            bg_sb = wpool.tile([128, CPC * GC], F32, name="bg_sb")
            ba_sb = wpool.tile([128, CPC * HC], F32, name="ba_sb")
            gsc_sb = wpool.tile([128, CPC], F32, name="gsc_sb")
            wproj_sb = wpool.tile([128, EC * HC * 128], BF, name="wproj_sb")
            bproj_sb = wpool.tile([128, HC], F32, name="bproj_sb")
            ident_sb = wpool.tile([128, 128], BF, name="ident_sb")
            ones_col = wpool.tile([128, 1], BF, name="ones_col")
            ones_row = wpool.tile([1, 128], BF, name="ones_row")
            zero_b = wpool.tile([128, B], BF, name="zero_b")

            h_st = [state.tile([128, HC, B], BF, name=f"h{c}") for c in range(CPC)]
            c_st = [state.tile([128, HC, B], F32, name=f"c{c}") for c in range(CPC)]
            # Whh*h + bg spill, written at end of step t, consumed in t+1
            gh = [state.tile([128, GC, B], BF, name=f"gh{c}") for c in range(CPC)]
            ext = state.tile([128, HC, B], BF, name="ext")
            y_acc = state.tile([128, HC, B], BF, name="y_acc")

            nc.sync.dma_start(wp_sb[:], wp_d[:])
            nc.sync.dma_start(wih_sb[:], wih_d[:])
            nc.sync.dma_start(whh_sb[:], whh_d[:])
            nc.sync.dma_start(wa_sb[:], wa_d[:])
            nc.sync.dma_start(w1n_sb[:], w1n_d[:])
            nc.sync.dma_start(bp_sb[:], bp_d[:])
            nc.sync.dma_start(bg_sb[:], bg_d[:])
            nc.sync.dma_start(ba_sb[:], ba_d[:])
            nc.sync.dma_start(gsc_sb[:], gsc_d[:])
            nc.sync.dma_start(wproj_sb[:], wproj_d[:])
            nc.sync.dma_start(bproj_sb[:], bproj_d[:])
            nc.sync.dma_start(ident_sb[:], ident_d[:])
            nc.vector.memset(ones_col[:], 1.0)
            nc.vector.memset(ones_row[:], 1.0)
            nc.vector.memset(zero_b[:], 0.0)
            for c in range(CPC):
                nc.vector.memset(h_st[c][:], 0.0)
                nc.vector.memset(c_st[c][:], 0.0)
                # gh(t=0) = Whh @ 0 + bg = bg, broadcast over B
                for mg in range(GC):
                    nc.vector.tensor_scalar(
                        gh[c][:, mg], zero_b[:],
                        bg_sb[:, c * GC + mg:c * GC + mg + 1], None, ALU.add,
                    )
            nc.vector.memset(ext[:], 0.0)

            # DRAM staging for xs^T
            xsT_loc = dpool1.tile([TLOC, H, B], BF, name="xsT_loc")
            xsT = dpool1.tile([T, H, B], BF, name="xsT", addr_space="Shared")

            # ---- preamble: embedding gather + projection (sharded by t) --
            with (
                tc.tile_pool(name="prepool", bufs=3) as pre,
                tc.tile_pool(name="preps", bufs=4, space="PSUM") as preps,
                tc.tile_pool(name="preps2", bufs=2, space="PSUM") as preps2,
            ):
                for g in range(NGRP):
                    embT = [
                        pre.tile([128, 512], BF, tag="embT", name=f"embT{g}_{k}")
                        for k in range(EC)
                    ]
                    for tt in range(4):
                        j = g * 4 + tt
                        idx = pre.tile([128, 1], I32, tag="idx", name=f"idx{j}")
                        nc.sync.dma_start(idx[:], tok_d[j])
                        gt = pre.tile([128, E], BF, tag="gt", name=f"gt{j}")
                        nc.gpsimd.indirect_dma_start(
                            out=gt[:],
                            out_offset=None,
                            in_=emb_d[:],
                            in_offset=bass.IndirectOffsetOnAxis(ap=idx[:, 0:1], axis=0),
                        )
                        for k in range(EC):
                            tp = preps.tile([128, 128], BF, tag="tp", name=f"tp{j}_{k}")
                            nc.tensor.transpose(
                                out=tp[:], in_=gt[:, k * 128:(k + 1) * 128],
                                identity=ident_sb[:],
                            )
                            nc.vector.tensor_copy(
                                embT[k][:, tt * 128:(tt + 1) * 128], tp[:]
                            )
                    for m in range(HC):
                        ps_x = preps2.tile([128, 512], F32, tag="psx", name=f"psx{g}_{m}")
                        for k in range(EC):
                            nc.tensor.matmul(
                                ps_x[:],
                                wproj_sb[:, (k * HC + m) * 128:(k * HC + m + 1) * 128],
                                embT[k][:],
                                start=(k == 0),
                                stop=(k == EC - 1),
                            )
                        xsg = pre.tile([128, 512], BF, tag="xsg", name=f"xsg{g}_{m}")
                        nc.scalar.activation(
                            xsg[:], ps_x[:], AF.Identity, bias=bproj_sb[:, m:m + 1]
                        )
                        nc.sync.dma_start(
                            xsT_loc[2 * g, m * 128:(m + 1) * 128, :], xsg[:, 0:B]
                        )
                        nc.sync.dma_start(
                            xsT_loc[2 * g + 1, m * 128:(m + 1) * 128, :], xsg[:, B:2 * B]
                        )

            nc.gpsimd.collective_compute(
                "AllGather",
                ALU.bypass,
                ins=[xsT_loc.opt()],
                outs=[xsT.opt()],
                replica_groups=RG,
            )

            # ---- the scan -----------------------------------------------
            with (
                tc.tile_pool(name="workpf", bufs=2) as workpf,
                tc.tile_pool(name="work", bufs=1) as work,
                tc.tile_pool(name="gq", bufs=1) as gqp,
                tc.tile_pool(name="sm", bufs=2) as smp,
                tc.tile_pool(name="ps_pg", bufs=3, space="PSUM") as ps_pg,
                tc.tile_pool(name="ps_pp", bufs=2, space="PSUM") as ps_pp,
                tc.tile_pool(name="ps_ss", bufs=1, space="PSUM") as ps_ss,
                tc.tile_pool(name="ps_pb", bufs=1, space="PSUM") as ps_pb,
            ):
                # GPSIMD (Pool) rejects TensorScalarPtr in codegen; keep
                # pointwise chains on DVE until Pool op support is probed.
                ENG = [nc.vector, nc.vector]
                for t in range(t_steps):
                    xt = workpf.tile([128, HC, B], BF, tag="xt", name=f"xt{t}")
                    nc.sync.dma_start(
                        xt[:], xsT[t].rearrange("(k p) b -> p k b", p=128)
                    )
                    # x~ = x_t + 0.3 * ext
                    xe = work.tile([128, HC, B], BF, tag="xe", name=f"xe{t}")
                    nc.vector.scalar_tensor_tensor(
                        xe[:], ext[:], 0.3, xt[:], ALU.mult, ALU.add
                    )

                    sbfs, msbfs, ps_list = [], [], []
                    st_t = ps_ss.tile([64, 2, B], F32, tag="ss", name=f"ss{t}")
                    pb_t = ps_pb.tile([128, 2, B], F32, tag="pb", name=f"pb{t}")
                    for c in range(CPC):
                        # ---- perception matmul + ReLU + LN stats --------
                        p_t = work.tile([128, HC, B], BF, tag=f"p{c}", name=f"p{t}_{c}")
                        p2 = work.tile([128, HC, B], BF, tag=f"p2{c}", name=f"p2{t}_{c}")
                        for pr in range(2):
                            pp = ps_pp.tile([128, 2, B], F32, tag="pp", name=f"pp{t}_{c}_{pr}")
                            for jj in range(2):
                                m = 2 * pr + jj
                                for k in range(HC):
                                    col = ((c * HC + k) * HC + m) * 128
                                    nc.tensor.matmul(
                                        pp[:, jj], wp_sb[:, col:col + 128], xe[:, k],
                                        start=(k == 0), stop=(k == HC - 1),
                                    )
                                nc.scalar.activation(
                                    p_t[:, m], pp[:, jj], AF.Relu,
                                    bias=bp_sb[:, c * HC + m:c * HC + m + 1],
                                )
                        ENG[c].tensor_mul(p2[:], p_t[:], p_t[:])
                        for m in range(HC):
                            nc.tensor.matmul(
                                st_t[0:1, c, :], ones_col[:], p_t[:, m],
                                start=(m == 0), stop=(m == HC - 1),
                            )
                            nc.tensor.matmul(
                                st_t[32:33, c, :], ones_col[:], p2[:, m],
                                start=(m == 0), stop=(m == HC - 1),
                            )
                        # ---- LN smalls: rstd via bit-trick + Newton -----
                        eng = ENG[c]
                        mu = smp.tile([1, B], F32, tag="mu", name=f"mu{t}_{c}")
                        vpe = smp.tile([1, B], F32, tag="vpe", name=f"vpe{t}_{c}")
                        musq = smp.tile([1, B], F32, tag="musq", name=f"musq{t}_{c}")
                        v_ = smp.tile([1, B], F32, tag="v", name=f"v{t}_{c}")
                        bi_ = smp.tile([1, B], I32, tag="bi", name=f"bi{t}_{c}")
                        y_ = smp.tile([1, B], F32, tag="y", name=f"y{t}_{c}")
                        y2_ = smp.tile([1, B], F32, tag="y2", name=f"y2{t}_{c}")
                        t_ = smp.tile([1, B], F32, tag="t", name=f"t{t}_{c}")
                        w_ = smp.tile([1, B], F32, tag="w", name=f"w{t}_{c}")
                        s_bf = smp.tile([1, B], BF, tag="sbf", name=f"sbf{t}_{c}")
                        ms_bf = smp.tile([1, B], BF, tag="msbf", name=f"msbf{t}_{c}")
                        # st/pb live in PSUM, which GPSIMD cannot read: those
                        # two reads stay on DVE regardless of cell engine.
                        nc.vector.tensor_scalar_mul(mu[:], st_t[0:1, c, :], 1.0 / H)
                        nc.vector.tensor_scalar(
                            vpe[:], st_t[32:33, c, :], 1.0 / H, LN_EPS, ALU.mult, ALU.add
                        )
                        eng.tensor_mul(musq[:], mu[:], mu[:])
                        eng.tensor_sub(v_[:], vpe[:], musq[:])
                        # y0 bits = 0x5f3759df - (i>>1) = ((i>>1) ^ -1) + 0x5f3759e0
                        eng.tensor_scalar(
                            bi_[:], v_[:].bitcast(I32), 1, -1,
                            ALU.logical_shift_right, ALU.bitwise_xor,
                        )
                        eng.tensor_scalar(
                            y_[:].bitcast(I32), bi_[:], 0x5F3759E0, None, ALU.add
                        )
                        for _ in range(NEWTON_ITERS):
                            eng.tensor_mul(y2_[:], y_[:], y_[:])
                            eng.tensor_mul(t_[:], y2_[:], v_[:])
                            eng.tensor_scalar(w_[:], t_[:], -0.5, 1.5, ALU.mult, ALU.add)
                            eng.tensor_mul(y_[:], y_[:], w_[:])
                        eng.tensor_copy(s_bf[:], y_[:])
                        eng.tensor_mul(ms_bf[:], mu[:], y_[:])
                        # broadcast rstd across partitions via rank-1 matmul
                        nc.tensor.matmul(
                            pb_t[:, c, :], ones_row[:], s_bf[:], start=True, stop=True
                        )
                        sb_bf = work.tile([128, B], BF, tag=f"sbb{c}", name=f"sbb{t}_{c}")
                        if c == 0:
                            nc.vector.tensor_copy(sb_bf[:], pb_t[:, c, :])
                        else:
                            nc.scalar.copy(sb_bf[:], pb_t[:, c, :])
                        p_s = work.tile([128, HC, B], BF, tag=f"psld{c}", name=f"psld{t}_{c}")
                        for m in range(HC):
                            eng.tensor_mul(p_s[:, m], p_t[:, m], sb_bf[:])
                        sbfs.append(s_bf)
                        msbfs.append(ms_bf)
                        ps_list.append(p_s)

                    for c in range(CPC):
                        p_s = ps_list[c]
                        ms_bf = msbfs[c]
                        eng = ENG[c]
                        # ---- gates: identity-add of gh + Wih part -------
                        gq = [
                            gqp.tile([128, HC, B], BF, tag=f"gq{c}_{gi}",
                                     name=f"gq{t}_{c}_{gi}")
                            for gi in range(4)
                        ]
                        for gi in range(4):
                            func = AF.Tanh if gi == 2 else AF.Sigmoid
                            for j2 in range(2):
                                pg = ps_pg.tile(
                                    [128, 2, B], F32, tag="pg", name=f"pg{t}_{c}_{gi}_{j2}"
                                )
                                for jj in range(2):
                                    j = 2 * j2 + jj
                                    mg = gi * HC + j
                                    nc.tensor.matmul(
                                        pg[:, jj], ident_sb[:], gh[c][:, mg],
                                        start=True, stop=False,
                                    )
                                    for k in range(HC):
                                        col = ((c * HC + k) * GC + mg) * 128
                                        nc.tensor.matmul(
                                            pg[:, jj], wih_sb[:, col:col + 128],
                                            p_s[:, k], start=False, stop=False,
                                        )
                                    col1 = (c * GC + mg) * 128
                                    nc.tensor.matmul(
                                        pg[:, jj], w1n_sb[0:1, col1:col1 + 128],
                                        ms_bf[:], start=False, stop=True,
                                    )
                                nc.scalar.activation(
                                    gq[gi][:, 2 * j2:2 * j2 + 2, :], pg[:], func
                                )
                        # ---- LSTM pointwise -----------------------------
                        t1 = gqp.tile([128, HC, B], BF, tag=f"t1{c}", name=f"t1{t}_{c}")
                        t2 = gqp.tile([128, HC, B], F32, tag=f"t2{c}", name=f"t2{t}_{c}")
                        eng.tensor_mul(t1[:], gq[0][:], gq[2][:])
                        eng.tensor_mul(t2[:], gq[1][:], c_st[c][:])
                        eng.tensor_add(c_st[c][:], t1[:], t2[:])
                        tc_ = gqp.tile([128, HC, B], BF, tag=f"tc{c}", name=f"tc{t}_{c}")
                        nc.scalar.activation(tc_[:], c_st[c][:], AF.Tanh)
                        eng.tensor_mul(h_st[c][:], gq[3][:], tc_[:])

                    a_sb = []
                    for c in range(CPC):
                        # ---- association ------------------------------
                        a_ = work.tile([128, HC, B], BF, tag=f"a{c}", name=f"a{t}_{c}")
                        for pr in range(2):
                            pa = ps_pp.tile([128, 2, B], F32, tag="pp", name=f"pa{t}_{c}_{pr}")
                            for jj in range(2):
                                m = 2 * pr + jj
                                for k in range(HC):
                                    col = ((c * HC + k) * HC + m) * 128
                                    nc.tensor.matmul(
                                        pa[:, jj], wa_sb[:, col:col + 128],
                                        h_st[c][:, k], start=(k == 0), stop=(k == HC - 1),
                                    )
                                nc.scalar.activation(
                                    a_[:, m], pa[:, jj], AF.Tanh,
                                    bias=ba_sb[:, c * HC + m:c * HC + m + 1],
                                )
                        a_sb.append(a_)
                    # y = sum_c gsc_c * a_c  (bf16)
                    nc.vector.tensor_scalar(
                        y_acc[:], a_sb[0][:], gsc_sb[:, 0:1], None, ALU.mult
                    )
                    nc.vector.scalar_tensor_tensor(
                        y_acc[:], a_sb[1][:], gsc_sb[:, 1:2], y_acc[:],
                        ALU.mult, ALU.add,
                    )

                    # ---- cross-cell mean via AllReduce ------------------
                    ar_i = dpool2.tile([H, B], BF, tag="ari", name=f"ari{t}")
                    ar_o = dpool2.tile(
                        [H, B], BF, tag="aro", name=f"aro{t}", addr_space="Shared"
                    )
                    nc.sync.dma_start(
                        ar_i.rearrange("(k p) b -> p k b", p=128), y_acc[:]
                    )
                    if os.environ.get("BASS_NOAR") == "1":
                        ar_o = ar_i  # timing probe: skip the collective
                    elif os.environ.get("BASS_AGRED") == "1":
                        # AllGather partials + local reduce (AG floor < AR floor)
                        ag_o = dpool2.tile(
                            [NCORES * H, B], BF, tag="ago", name=f"ago{t}",
                            addr_space="Shared",
                        )
                        nc.gpsimd.collective_compute(
                            "AllGather",
                            ALU.bypass,
                            ins=[ar_i.opt()],
                            outs=[ag_o.opt()],
                            replica_groups=RG,
                        )
                        allp = work.tile([128, NCORES, HC, B], BF, tag="allp",
                                         name=f"allp{t}")
                        nc.sync.dma_start(
                            allp[:],
                            ag_o.rearrange("(r k p) b -> p r k b", p=128, r=NCORES),
                        )
                        s4 = work.tile([128, 4, HC, B], BF, tag="s4", name=f"s4{t}")
                        s2 = work.tile([128, 2, HC, B], BF, tag="s2", name=f"s2{t}")
                        nc.vector.tensor_add(s4[:], allp[:, 0:4], allp[:, 4:8])
                        nc.vector.tensor_add(s2[:], s4[:, 0:2], s4[:, 2:4])
                        tgt = ext if t < t_steps - 1 else y_acc
                        nc.vector.tensor_add(tgt[:], s2[:, 0], s2[:, 1])
                        nc.sync.dma_start(
                            out_d[t].rearrange("(k p) b -> p k b", p=128), tgt[:]
                        )
                        ar_o = None
                    else:
                        nc.gpsimd.collective_compute(
                            "AllReduce",
                            ALU.add,
                            ins=[ar_i.opt()],
                            outs=[ar_o.opt()],
                            replica_groups=RG,
                        )

                    # ---- Whh prefetch for step t+1 (fills the AR window)
                    if t < t_steps - 1:
                        for c in range(CPC):
                            for pr in range(GC // 2):
                                pw = ps_pg.tile(
                                    [128, 2, B], F32, tag="pg", name=f"pw{t}_{c}_{pr}"
                                )
                                for jj in range(2):
                                    mg = 2 * pr + jj
                                    for k in range(HC):
                                        col = ((c * HC + k) * GC + mg) * 128
                                        nc.tensor.matmul(
                                            pw[:, jj], whh_sb[:, col:col + 128],
                                            h_st[c][:, k],
                                            start=(k == 0), stop=(k == HC - 1),
                                        )
                                    nc.vector.tensor_scalar(
                                        gh[c][:, 2 * pr + jj], pw[:, jj],
                                        bg_sb[:, c * GC + mg:c * GC + mg + 1],
                                        None, ALU.add,
                                    )

                    if ar_o is not None:
                        if t < t_steps - 1:
                            nc.sync.dma_start(
                                ext[:], ar_o.rearrange("(k p) b -> p k b", p=128)
                            )
                        nc.sync.dma_start(out_d[t], ar_o[:])

    nc.compile()
    return nc


def prepare_inputs(tokens, emb, Wproj, bproj, Wp, bp, ln_g, ln_b,
                   Wih, bih, Whh, bhh, Wa, ba, gate_logit):
    """Host-side parameter prep + per-core sharding. Returns in_maps."""
    tokens = np.asarray(tokens).astype(np.int32)
    emb = np.asarray(emb, dtype=np.float32).copy()
    emb[0] = 0.0  # padding_idx
    emb_bf = emb.astype(BF16)

    Wproj = np.asarray(Wproj, np.float32)
    bproj = np.asarray(bproj, np.float32)
    Wp = np.asarray(Wp, np.float32)
    bp = np.asarray(bp, np.float32)
    ln_g = np.asarray(ln_g, np.float32)
    ln_b = np.asarray(ln_b, np.float32)
    Wih = np.asarray(Wih, np.float32)
    bih = np.asarray(bih, np.float32)
    Whh = np.asarray(Whh, np.float32)
    bhh = np.asarray(bhh, np.float32)
    Wa = np.asarray(Wa, np.float32)
    ba = np.asarray(ba, np.float32)
    gate_logit = np.asarray(gate_logit, np.float32)

    # Fold the LN affine (g, b) into the input-hidden weights / gate bias.
    Wih_g = Wih * ln_g[:, None, :]                       # [C, 4H, H]
    bg = bih + np.einsum("cgh,ch->cg", Wih, ln_b) + bhh  # [C, 4H]
    w1n = -Wih_g.sum(-1)                                 # [C, 4H]
    gsc = 1.0 / (1.0 + np.exp(-gate_logit)) / C          # [C]

    wproj_p = _pack_lhsT(Wproj).astype(BF16)
    bproj_p = _pack_bias(bproj[None, :])                 # [128, 4]
    ident = np.eye(128, dtype=np.float32).astype(BF16)

    in_maps = []
    for i in range(NCORES):
        cs = slice(CPC * i, CPC * (i + 1))
        wp_p = np.concatenate([_pack_lhsT(Wp[c]) for c in range(cs.start, cs.stop)], 1)
        wih_p = np.concatenate(
            [_pack_lhsT(np.ascontiguousarray(Wih_g[c].T)) for c in range(cs.start, cs.stop)], 1
        )
        whh_p = np.concatenate(
            [_pack_lhsT(np.ascontiguousarray(Whh[c].T)) for c in range(cs.start, cs.stop)], 1
        )
        wa_p = np.concatenate([_pack_lhsT(Wa[c]) for c in range(cs.start, cs.stop)], 1)

        t0 = TLOC * i
        tok_core = np.ascontiguousarray(
            tokens[:, t0:t0 + TLOC].T.reshape(NGRP * 4, 128, 1)
        )

        in_maps.append({
            "emb": emb_bf,
            "tok": tok_core,
            "wproj": wproj_p,
            "bproj": bproj_p,
            "wp": wp_p.astype(BF16),
            "wih": wih_p.astype(BF16),
            "whh": whh_p.astype(BF16),
            "wa": wa_p.astype(BF16),
            "w1n": w1n[cs].reshape(1, -1).astype(BF16),
            "bp": _pack_bias(bp[cs]),
            "bg": _pack_bias(bg[cs]),
            "ba": _pack_bias(ba[cs]),
            "gsc": np.broadcast_to(gsc[cs], (128, CPC)).astype(np.float32).copy(),
            "ident": ident,
        })
    return in_maps


_CACHE = {}


def run(inputs: dict, t_steps: int = T, trace: bool = False):
    key = t_steps
    if key not in _CACHE:
        _CACHE[key] = build_program(t_steps)
    nc = _CACHE[key]
    in_maps = prepare_inputs(**inputs)
    res = run_bass_kernel_spmd(nc, in_maps, list(range(NCORES)), trace=trace)
    ysT = np.asarray(res.results[0]["out"], dtype=np.float32)  # [t_steps, H, B]
    out = np.ascontiguousarray(ysT.transpose(2, 0, 1))  # [B, t_steps, H]
    return out, res


def kernel(**inputs) -> np.ndarray:
    out, _ = run(inputs, T)
    return out


def run_timed(inputs: dict, t_steps: int = T, n_iters: int = 3):
    """Replicates bass2jax.run_bass_via_pjrt's multi-core path but keeps the
    jitted executable and device-resident inputs so repeat calls measure the
    on-device execution time (plus dispatch) rather than NEFF compile or
    host->device transfer."""
    import time
    import jax
    from jax.sharding import Mesh, PartitionSpec
    from jax.experimental.shard_map import shard_map
    from concourse import bass2jax, mybir as _mb

    key = t_steps
    if key not in _CACHE:
        _CACHE[key] = build_program(t_steps)
    nc = _CACHE[key]
    in_maps = prepare_inputs(**inputs)

    bass2jax.install_neuronx_cc_hook()
    part_name = nc.partition_id_tensor.name if nc.partition_id_tensor else None
    in_names, out_names, out_avals, zero_outs = [], [], [], []
    for alloc in nc.m.functions[0].allocations:
        if not isinstance(alloc, _mb.MemoryLocationSet):
            continue
        name = alloc.memorylocations[0].name
        if alloc.kind == "ExternalInput":
            if name != part_name:
                in_names.append(name)
        elif alloc.kind == "ExternalOutput":
            out_names.append(name)
            out_avals.append(
                jax.core.ShapedArray(alloc.tensor_shape, _mb.dt.np(alloc.dtype))
            )
            zero_outs.append(
                np.zeros(alloc.tensor_shape, dtype=_mb.dt.np(alloc.dtype))
            )
    n_params = len(in_names)
    all_names = in_names + out_names
    if part_name is not None:
        all_names.append(part_name)

    def _body(*args):
        operands = list(args)
        if part_name is not None:
            operands.append(bass2jax.partition_id_tensor())
        outs = bass2jax._bass_exec_p.bind(
            *operands,
            out_avals=tuple(out_avals),
            in_names=tuple(all_names),
            out_names=tuple(out_names),
            lowering_input_output_aliases=(),
            sim_require_finite=True,
            sim_require_nnan=True,
            nc=nc,
        )
        return tuple(outs)

    devices = jax.devices()[:NCORES]
    mesh = Mesh(np.asarray(devices), ("core",))
    n_outs = len(out_names)
    sharded = jax.jit(
        shard_map(
            _body, mesh=mesh,
            in_specs=(PartitionSpec("core"),) * (n_params + n_outs),
            out_specs=(PartitionSpec("core"),) * n_outs,
            check_rep=False,
        ),
        keep_unused=True,
    )
    concat_in = [
        np.concatenate([np.asarray(in_maps[c][nm]) for c in range(NCORES)], axis=0)
        for nm in in_names
    ]
    concat_zeros = [
        np.zeros((NCORES * z.shape[0], *z.shape[1:]), z.dtype) for z in zero_outs
    ]
    sh = jax.sharding.NamedSharding(mesh, PartitionSpec("core"))
    dev_in = [jax.device_put(a, sh) for a in concat_in]
    dev_zero = [jax.device_put(a, sh) for a in concat_zeros]
    out_arrs = sharded(*dev_in, *dev_zero)  # warm-up / compile
    jax.block_until_ready(out_arrs)
    # pipeline n_iters calls without intermediate blocking to amortize the
    # axon dispatch round-trip; calls serialize on the devices.
    n_pipe = max(n_iters, 12)
    t0 = time.perf_counter()
    rs = [sharded(*dev_in, *dev_zero) for _ in range(n_pipe)]
    jax.block_until_ready(rs)
    per_call = (time.perf_counter() - t0) / n_pipe
    idx = out_names.index("out")
    ysT = np.asarray(out_arrs[idx]).reshape(NCORES, *out_avals[idx].shape)[0]
    out = np.ascontiguousarray(ysT.astype(np.float32).transpose(2, 0, 1))
    return out, per_call
